# revision 6
# baseline (speedup 1.0000x reference)
"""Trainium2 Bass kernel for nn_Logic_Learning_Model (logic-rule point-process
log-likelihood), restructured for speed.

Key differences vs the first-generation kernel:
- No per-partition-scalar (imm_src=PTR) tensor_scalar ops: those run at
  ~15 ns/elem on HW. Masks are built with DVE tensor_tensor against
  stride-0 broadcast threshold columns (exact 0/1) and with ScalarE
  Sigmoid(K*(tq - t)) with K=1e30 (saturates to exact 0/1 in fp32).
- Mask matmuls are flipped: mask is the lhsT (stationary), value quads are
  the rhs, so PSUM comes out query-major [q, role]. This eliminates the
  partition->free repack DMAs (82K 4-byte packets) of the old kernel.
- Query rows (head/pad/grid, and the tq-TOL variant with reference fp32
  rounding) are precomputed on the host and broadcast by one DMA per
  sample.
- The E (head-state) role needs no matmul for head queries: sh[idx] for
  head query i is states[s,4,i-1], already laid out [event, sample] in
  SBUF. Only the 256 grid queries use a mask matmul.
- Final reduction (sum of head logits, sum of exp(grid logits)) happens
  on-device via ones-matmuls; each core returns one scalar.

Sharding: pure data parallel, 32 samples per core on 8 cores; host sums
the 8 per-core scalars.
"""

import numpy as np

import concourse.bass as bass
import concourse.mybir as mybir
from concourse.tile import TileContext

F32 = mybir.dt.float32
BF16 = mybir.dt.bfloat16
I32 = mybir.dt.int32
U8 = mybir.dt.uint8

NCORES = 8
S = 32          # samples per core
E = 256         # events per predicate
EH = 128        # one partition tile of events
Q = 512         # queries: 255 head + 1 pad + 256 grid
QM = 1024       # tq cols: [fl(tq-0.1) | tq]
QT = 1280       # tqmain row width: [fl(tq-0.1) | tq | fl(t1-0.1)]
T_MAX = 76.8
RES = 0.3
TOL = 0.1
C1 = 38.4
C2 = 76.8
BIGK = 1.0e30   # sigmoid sharpness: saturates to exact 0/1 in fp32
ASGN = 22       # ~2/3 of samples: A-masks on ScalarE, interleaved

OP = mybir.AluOpType
ACTF = mybir.ActivationFunctionType


def bcast(ap, n=128):
    """0-stride partition broadcast view of a flat DRAM AP."""
    return bass.AP(ap.tensor, ap.offset, [[0, n]] + list(ap.ap))


def pk_in0(ap, colw):
    """[128, colw] view read twice -> [128, 2*colw] (outer stride 0)."""
    return bass.AP(ap.tensor, ap.offset, [list(ap.ap[0]), [0, 2], [1, colw]])


def pk_in1(ap, colw):
    """[128, 2] column pair -> [128, 2*colw] (inner stride 0)."""
    return bass.AP(ap.tensor, ap.offset, [list(ap.ap[0]), [1, 2], [0, colw]])


def pk4_in0(ap, colw):
    """[128, colw] view read 4x -> [128, 4*colw] (outer stride 0)."""
    return bass.AP(ap.tensor, ap.offset, [list(ap.ap[0]), [0, 4], [1, colw]])


def pk4_in1(ap, colw):
    """[128, 4] column quad -> [128, 4*colw] (inner stride 0)."""
    return bass.AP(ap.tensor, ap.offset, [list(ap.ap[0]), [1, 4], [0, colw]])


def col_bc(ap, colw):
    """[128, 1] column -> [128, colw] stride-0 broadcast."""
    return bass.AP(ap.tensor, ap.offset, [list(ap.ap[0]), [0, colw]])


def build_nc():
    from concourse.bacc import Bacc
    nc = Bacc(None, target_bir_lowering=False)
    times_d = nc.dram_tensor("times", [S, 5, E], F32, kind="ExternalInput")
    states_d = nc.dram_tensor("states", [S, 5, E], I32, kind="ExternalInput")
    base_d = nc.dram_tensor("base", [1], F32, kind="ExternalInput")
    weights_d = nc.dram_tensor("weights", [3], F32, kind="ExternalInput")
    tqmain_d = nc.dram_tensor("tqmain", [S, QT], F32, kind="ExternalInput")
    eye_d = nc.dram_tensor("eye32", [32, 32], F32, kind="ExternalInput")
    # host-pretransposed event data; see make_inputs_for_core for layout
    prep_d = nc.dram_tensor("prep", [128, 1280], F32, kind="ExternalInput")
    esrow_d = nc.dram_tensor("esrow1", [1, 128], F32, kind="ExternalInput")
    out_d = nc.dram_tensor("out", [1], F32, kind="ExternalOutput")
    import os
    dbg_d = None
    if os.environ.get("KDBG"):
        dbg_d = nc.dram_tensor("dbg", [128, 256], F32, kind="ExternalOutput")
        dbg2_d = nc.dram_tensor("dbg2", [128, 2380], F32,
                                kind="ExternalOutput")
        nc._dbg2 = dbg2_d

    with TileContext(nc) as tc:
        _build(tc, nc, times_d, states_d, base_d, weights_d, tqmain_d,
               eye_d, prep_d, esrow_d, out_d, dbg_d)
    nc.finalize()
    return nc


def _build(tc, nc, times_d, states_d, base_d, weights_d, tqmain_d,
           eye_d, prep_d, esrow_d, out_d, dbg_d=None):
    cp = tc.alloc_tile_pool(name="const", bufs=1)
    sp = tc.alloc_tile_pool(name="scratch", bufs=3)
    qp = tc.alloc_tile_pool(name="tq", bufs=4)
    mp = tc.alloc_tile_pool(name="mask", bufs=4)
    map_ = tc.alloc_tile_pool(name="maska", bufs=4)
    imp = tc.alloc_tile_pool(name="maski", bufs=4)
    pw = tc.alloc_tile_pool(name="psw", bufs=1, space="PSUM")
    pr = tc.alloc_tile_pool(name="prole", bufs=1, space="PSUM")

    # ---------------- phase 0: one dense load + batched prep ----------------
    prep = cp.tile([128, 1280], F32, tag="prep", name="t")
    nc.sync.dma_start(out=prep[:], in_=prep_d[:])

    # views into host-pretransposed prep (see make_inputs_for_core)
    T = {(a, kt): prep[:, (2 * a + kt) * S:(2 * a + kt + 1) * S]
         for a in range(5) for kt in range(2)}
    Tpk = {a: prep[:, 320 + ai * 2 * S:320 + (ai + 1) * 2 * S]
           for ai, a in enumerate((0, 1, 2, 4))}
    nKT = {(3, 0): prep[:, 576:608], (3, 1): prep[:, 608:640],
           (1, 0): prep[:, 640:672], (1, 1): prep[:, 672:704]}
    ST = {(a, kt): prep[:, 704 + (2 * a + kt) * S:704 + (2 * a + kt + 1) * S]
          for a in range(5) for kt in range(2)}
    shm1 = {kt: prep[:, 1024 + kt * S:1024 + (kt + 1) * S] for kt in range(2)}
    Tpk21 = prep[:, 1088:1216]   # [t2k0,t2k1,t1k0,t1k1] at 4s
    nKT0 = {kt: prep[:, 1216 + kt * S:1216 + (kt + 1) * S]
            for kt in range(2)}

    # base/weights broadcast columns
    wbbc = cp.tile([128, 4], F32, tag="wbbc", name="t")
    nc.vector.memset(wbbc[:], 0.0)
    nc.sync.dma_start(out=wbbc[:, 0:3], in_=bcast(weights_d[:]))
    nc.sync.dma_start(out=wbbc[:, 3:4], in_=bcast(base_d[:]))
    negw2 = cp.tile([128, 1], F32, tag="negw2", name="t")
    nc.vector.tensor_scalar(out=negw2[:], in0=wbbc[:, 2:3], scalar1=-1.0,
                            scalar2=None, op0=OP.mult)

    # batched exponentials / state masks
    ew = {}
    sm = {}
    for kt in range(2):
        def _exp(tag, src, scale, off):
            arg = sp.tile([EH, S], F32, tag=f"arg{tag}{kt}", name="t")
            nc.vector.tensor_scalar(out=arg[:], in0=src, scalar1=scale,
                                    scalar2=off, op0=OP.mult, op1=OP.add)
            e_t = cp.tile([EH, S], F32, tag=f"e{tag}{kt}", name="t")
            nc.scalar.activation(e_t[:], arg[:], ACTF.Exp)
            return e_t

        ew["w0", kt] = _exp("w0", T[0, kt], 1.0, -C2)        # e^{t0-C2}
        ew["c2t1", kt] = _exp("c2t1", T[1, kt], -1.0, C2)    # e^{C2-t1}
        ew["g1", kt] = _exp("g1", T[1, kt], 2.0, -2.0 * C1)  # e^{2(t1-C1)}
        ew["g2", kt] = _exp("g2", T[1, kt], 2.0, -2.0 * C2)
        ew["v21", kt] = _exp("v21", T[2, kt], 1.0, -C1)
        ew["v22", kt] = _exp("v22", T[2, kt], 1.0, -C2)
        ew["v31", kt] = _exp("v31", T[3, kt], 1.0, -C1)
        ew["v32", kt] = _exp("v32", T[3, kt], 1.0, -C2)

        for a, val, tag in ((0, 1, "s0"), (1, 1, "s1"), (2, 1, "s2"), (3, 0, "s3")):
            m = cp.tile([EH, S], F32, tag=f"{tag}{kt}", name="t")
            nc.vector.tensor_scalar(out=m[:], in0=ST[a, kt], scalar1=val,
                                    scalar2=None, op0=OP.is_equal)
            sm[tag, kt] = m

        # [t3 <= C1]: zero the v3C1 entries no C1-block query can select,
        # keeping the C1-version value range bounded.
        m31 = cp.tile([EH, S], F32, tag=f"m31{kt}", name="t")
        nc.vector.tensor_scalar(out=m31[:], in0=T[3, kt], scalar1=C1,
                                scalar2=None, op0=OP.is_le)
        sm["m31", kt] = m31

    def dekker(dst, blk0, src32, tmp_tag):
        """bf16 (hi, lo) of src32 [128, S] into dst col blocks blk0, blk0+1."""
        hi = dst[:, blk0 * S:(blk0 + 1) * S]
        lo = dst[:, (blk0 + 1) * S:(blk0 + 2) * S]
        nc.vector.tensor_copy(out=hi, in_=src32[:])
        tmp = sp.tile([EH, S], F32, tag=tmp_tag, name="t")
        nc.vector.tensor_copy(out=tmp[:], in_=hi)
        nc.vector.tensor_tensor(out=lo, in0=src32[:], in1=tmp[:],
                                op=OP.subtract)

    # w0 pairs: [128, 2S], blocks [hi | lo]
    w0pair = {}
    for kt in range(2):
        w0 = sp.tile([EH, S], F32, tag=f"w0m{kt}", name="t")
        nc.vector.tensor_tensor(out=w0[:], in0=ew["w0", kt][:],
                                in1=sm["s0", kt][:], op=OP.mult)
        pair = cp.tile([EH, 2 * S], BF16, tag=f"w0pair{kt}", name="t")
        dekker(pair, 0, w0, f"w0tmp{kt}")
        w0pair[kt] = pair

    # vB / vC quads [128, 4S]: blocks [v1h | v1l | v2h | v2l]
    vB = {}
    vC = {}
    for kt in range(2):
        q_b = cp.tile([EH, 4 * S], BF16, tag=f"vB{kt}", name="t")
        q_c = cp.tile([EH, 4 * S], BF16, tag=f"vC{kt}", name="t")
        for ver, (e2tag, e3tag) in enumerate((("v21", "v31"), ("v22", "v32"))):
            v2 = sp.tile([EH, S], F32, tag=f"v2m{kt}{ver}", name="t")
            nc.vector.tensor_tensor(out=v2[:], in0=ew[e2tag, kt][:],
                                    in1=sm["s2", kt][:], op=OP.mult)
            dekker(q_b, 2 * ver, v2, f"dkb{kt}{ver}")
            v3 = sp.tile([EH, S], F32, tag=f"v3m{kt}{ver}", name="t")
            nc.vector.tensor_tensor(out=v3[:], in0=ew[e3tag, kt][:],
                                    in1=sm["s3", kt][:], op=OP.mult)
            if ver == 0:
                nc.vector.tensor_tensor(out=v3[:], in0=v3[:],
                                        in1=sm["m31", kt][:], op=OP.mult)
            dekker(q_c, 2 * ver, v3, f"dkc{kt}{ver}")
        vB[kt] = q_b
        vC[kt] = q_c

    # dsh (bf16) [128, S]: sh_j - sh_{j-1 (wrap)} (shifted sh from prep)
    dsh = {}
    for kt in range(2):
        d = cp.tile([EH, S], BF16, tag=f"dsh{kt}", name="t")
        nc.vector.tensor_tensor(out=d[:], in0=ST[4, kt], in1=shm1[kt],
                                op=OP.subtract)
        dsh[kt] = d

    # escrow [1, 128]: head cols 1.0; grid cols 1 - 2*sh255 (host-built)
    escrow = cp.tile([1, 128], F32, tag="escrow", name="t")
    nc.sync.dma_start(out=escrow[:], in_=esrow_d[:])

    ones1 = cp.tile([1, 128], F32, tag="ones1", name="t")
    nc.vector.memset(ones1[:], 1.0)
    onescol = cp.tile([128, 1], F32, tag="onescol", name="t")
    nc.vector.memset(onescol[:], 1.0)

    eye32 = cp.tile([32, 32], F32, tag="eye32", name="t")
    nc.sync.dma_start(out=eye32[:], in_=eye_d[:])
    tqrows = cp.tile([32, Q], F32, tag="tqrows", name="t")
    nc.sync.dma_start(out=tqrows[:], in_=tqmain_d[:, Q:QM])

    # ------------- merged main loop (groups of 4 samples) -------------
    # Per sample: inner mask + all role masks + inner/BCDE matmuls. After
    # each group's inner matmuls: per-group gA assembly, then the group's
    # A-role matmuls. This keeps DVE/ScalarE/PE busy concurrently instead
    # of a serial inner-mask prefix.
    # One PSUM bank per role (matmul start=True clears has_written for the
    # whole 2KB zero region -> one open accumulation group per bank).
    # ABCD: col = s*16 + qt*4; E: col = s*4 + qt.
    psw = pw.tile([128, 4 * S], F32, tag="psw", name="t")
    psA = pr.tile([128, 512], F32, tag="psA", name="t")
    psB = pr.tile([128, 512], F32, tag="psB", name="t")
    psC = pr.tile([128, 512], F32, tag="psC", name="t")
    psD = pr.tile([128, 512], F32, tag="psD", name="t")
    psE = pr.tile([128, 128], F32, tag="psE", name="t")

    wst = cp.tile([128, 4 * S], F32, tag="wst", name="t")
    gA = {kt: cp.tile([EH, 4 * S], BF16, tag=f"gA{kt}", name="t")
          for kt in range(2)}

    def build_masks(s, tq4, j, inner=True):
        tqm = tq4[:, j * QM:(j + 1) * QM]
        im = None
        mEg = mp.tile([EH, Q], BF16, tag="mEg", name="t")
        nc.vector.tensor_tensor(out=mEg[:], in0=pk_in0(tqm[:, 768:QM], 256),
                                in1=pk_in1(Tpk[4][:, 2 * s:2 * s + 2], 256),
                                op=OP.is_gt)
        mCD = {}
        for kt in range(2):
            cd = mp.tile([EH, QM], BF16, tag=f"mCD{kt}", name="t")
            nc.scalar.activation(cd[:], tqm, ACTF.Sigmoid,
                                 bias=nKT[3, kt][:, s:s + 1], scale=BIGK)
            mCD[kt] = cd
        if s % 2 == 0:  # A on ScalarE; B alone on DVE
            mB = mp.tile([EH, QM], BF16, tag="mB", name="t")
            nc.vector.tensor_tensor(out=mB[:], in0=pk_in0(tqm[:, 0:Q], Q),
                                    in1=pk_in1(Tpk[2][:, 2 * s:2 * s + 2], Q),
                                    op=OP.is_gt)
            mA = map_.tile([EH, QM], BF16, tag="mA", name="t")
            for kt in range(2):
                nc.scalar.activation(mA[:, kt * Q:(kt + 1) * Q], tqm[:, 0:Q],
                                     ACTF.Sigmoid, bias=nKT[1, kt][:, s:s + 1],
                                     scale=BIGK)
        else:  # one packed DVE op: [mB-kt0 | mB-kt1 | mA-kt0 | mA-kt1]
            ba = map_.tile([EH, 2 * QM], BF16, tag="mBA", name="t")
            nc.vector.tensor_tensor(out=ba[:], in0=pk4_in0(tqm[:, 0:Q], Q),
                                    in1=pk4_in1(Tpk21[:, 4 * s:4 * s + 4], Q),
                                    op=OP.is_gt)
            mB = ba[:, 0:QM]
            mA = ba[:, QM:2 * QM]
        return im, mB, mEg, mCD, mA

    def inner_mms(s, im):
        for jkt in range(2):
            for ikt in range(2):
                nc.tensor.matmul(
                    psw[:, 4 * s + 2 * jkt:4 * s + 2 * jkt + 2],
                    im[:, ikt * E + jkt * EH:ikt * E + (jkt + 1) * EH],
                    w0pair[ikt][:, s::S][:, 0:2],
                    start=(ikt == 0), stop=(ikt == 1))

    def bcde_mms(s, mB, mEg, mCD):
        for qt in range(4):
            q0 = qt * EH
            c4 = s * 16 + qt * 4
            for kt in range(2):
                nc.tensor.matmul(psB[:, c4:c4 + 4],
                                 mB[:, kt * Q + q0:kt * Q + q0 + EH],
                                 vB[kt][:, s::S][:, 0:4],
                                 start=(kt == 0), stop=(kt == 1))
            for kt in range(2):
                nc.tensor.matmul(psC[:, c4:c4 + 4],
                                 mCD[kt][:, q0:q0 + EH],
                                 vC[kt][:, s::S][:, 0:4],
                                 start=(kt == 0), stop=(kt == 1))
            for kt in range(2):
                nc.tensor.matmul(psD[:, c4:c4 + 4],
                                 mCD[kt][:, Q + q0:Q + q0 + EH],
                                 vC[kt][:, s::S][:, 0:4],
                                 start=(kt == 0), stop=(kt == 1))
            if qt >= 2:
                for kt in range(2):
                    nc.tensor.matmul(
                        psE[:, s * 4 + qt:s * 4 + qt + 1],
                        mEg[:, kt * 256 + (qt - 2) * EH:
                            kt * 256 + (qt - 1) * EH],
                        dsh[kt][:, s:s + 1], start=(kt == 0), stop=(kt == 1))

    def a_mms(s, mA):
        for qt in range(4):
            q0 = qt * EH
            c4 = s * 16 + qt * 4
            for kt in range(2):
                nc.tensor.matmul(psA[:, c4:c4 + 4],
                                 mA[:, kt * Q + q0:kt * Q + q0 + EH],
                                 gA[kt][:, s::S][:, 0:4],
                                 start=(kt == 0), stop=(kt == 1))

    def phase2_half(h):
        """gA columns for samples 16h..16h+15 (batched FD=16)."""
        c0 = 64 * h
        s0 = 16 * h
        nc.scalar.copy(wst[:, c0:c0 + 64], psw[:, c0:c0 + 64])
        for kt in range(2):
            wh = sp.tile([EH, 16], F32, tag=f"wh{kt}", name="t", bufs=3)
            nc.vector.tensor_tensor(out=wh[:],
                                    in0=wst[:, c0 + 2 * kt:c0 + 64:4],
                                    in1=wst[:, c0 + 2 * kt + 1:c0 + 64:4],
                                    op=OP.add)
            nc.vector.tensor_tensor(out=wh[:], in0=wh[:],
                                    in1=ew["c2t1", kt][:, s0:s0 + 16],
                                    op=OP.mult)
            for ver, etag in enumerate(("g1", "g2")):
                g32 = sp.tile([EH, 16], F32, tag=f"g32{kt}{ver}", name="t",
                              bufs=3)
                nc.vector.tensor_tensor(out=g32[:],
                                        in0=ew[etag, kt][:, s0:s0 + 16],
                                        in1=wh[:], op=OP.mult)
                nc.vector.tensor_tensor(out=g32[:], in0=g32[:],
                                        in1=sm["s1", kt][:, s0:s0 + 16],
                                        op=OP.mult)
                blk0 = 2 * ver
                hi = gA[kt][:, blk0 * S + s0:blk0 * S + s0 + 16]
                lo = gA[kt][:, (blk0 + 1) * S + s0:(blk0 + 1) * S + s0 + 16]
                nc.vector.tensor_copy(out=hi, in_=g32[:])
                tmp = sp.tile([EH, 16], F32, tag=f"dkg{kt}{ver}", name="t",
                              bufs=3)
                nc.vector.tensor_copy(out=tmp[:], in_=hi)
                nc.vector.tensor_tensor(out=lo, in0=g32[:], in1=tmp[:],
                                        op=OP.subtract)

    # pass A: inner masks + What matmuls (overlaps pass B via scheduler)
    for g in range(S // 4):
        tqi4 = qp.tile([EH, 4 * E], F32, tag="tqi4", name="t", bufs=2)
        nc.sync.dma_start(out=tqi4[:],
                          in_=bcast(tqmain_d[4 * g:4 * g + 4, QM:QT]))
        for j in range(4):
            s = 4 * g + j
            tqi = tqi4[:, j * E:(j + 1) * E]
            im = imp.tile([EH, 2 * E], BF16, tag="im", name="t")
            nc.vector.tensor_tensor(out=im[:], in0=pk_in0(tqi, E),
                                    in1=pk_in1(Tpk[0][:, 2 * s:2 * s + 2], E),
                                    op=OP.is_gt)
            inner_mms(s, im)
    phase2_half(0)
    phase2_half(1)

    # pass B: role masks + all role matmuls
    stage = cp.tile([128, 2176], F32, tag="stage", name="t")

    def stage_copy_half(h):
        c = 256 * h
        for i, ps in enumerate((psA, psB, psC, psD)):
            nc.scalar.copy(stage[:, i * 512 + c:i * 512 + c + 256],
                           ps[:, c:c + 256])
        nc.scalar.copy(stage[:, 2048 + 64 * h:2048 + 64 * h + 64],
                       psE[:, 64 * h:64 * h + 64])

    for g in range(S // 4):
        tq4 = qp.tile([EH, 4 * QM], F32, tag="tq4", name="t", bufs=4)
        nc.sync.dma_start(out=tq4[:],
                          in_=bcast(tqmain_d[4 * g:4 * g + 4, 0:QM]))
        for j in range(4):
            s = 4 * g + j
            _, mB, mEg, mCD, mA = build_masks(s, tq4, j, inner=False)
            bcde_mms(s, mB, mEg, mCD)
            a_mms(s, mA)
        if g == S // 8 - 1:
            stage_copy_half(0)
    stage_copy_half(1)

    pr.release()
    pw.release()

    # ---------------- phase 4: batched post-processing ----------------
    pf = tc.alloc_tile_pool(name="pfin", bufs=1, space="PSUM")

    def Rv(role, hilo):
        """[128, 4, 32, 1] view of one (role-bank, hilo) plane, (qt, s)."""
        v = stage[:, role * 512:(role + 1) * 512].rearrange(
            "p (s q r) -> p q s r", s=S, q=4, r=4)
        return v[:, :, :, hilo:hilo + 1]

    def Ev():  # E grid half -> [128, 2, 32, 1], (qt-2, s)
        v = stage[:, 2048:2176].rearrange("p (s q) -> p q s", s=S, q=4)
        return v[:, 2:4, :]

    # tqT [128 q, 128 (qt,s)] via transpose matmuls
    tqT_ps = pf.tile([128, 128], F32, tag="tqTps", name="t")
    for qt in range(4):
        nc.tensor.matmul(tqT_ps[:, 32 * qt:32 * (qt + 1)],
                         tqrows[:, qt * EH:(qt + 1) * EH], eye32[:],
                         start=True, stop=True)
    tqT = cp.tile([128, 128], F32, tag="tqT", name="t")
    nc.scalar.copy(tqT[:], tqT_ps[:])

    # esc psum [128, 128] = escrow broadcast down partitions
    esc_ps = pf.tile([128, 128], F32, tag="escps", name="t")
    nc.tensor.matmul(esc_ps[:], ones1[:], escrow[:], start=True, stop=True)

    def tmp(tag):
        return cp.tile([128, 128], F32, tag=tag, name="t")

    # hi+lo pair sums: quad cols per (s,qt) = [v1h, v1l, v2h, v2l]
    A1, A2 = tmp("A1"), tmp("A2")
    B1, B2 = tmp("B1"), tmp("B2")
    Cd1, Cd2 = tmp("Cd1"), tmp("Cd2")
    for dst, role, ver in ((A1, 0, 0), (A2, 0, 2), (B1, 1, 0), (B2, 1, 2)):
        nc.vector.tensor_tensor(out=dst[:], in0=Rv(role, ver),
                                in1=Rv(role, ver + 1), op=OP.add)
    # D - C per version: (D_h - C_h) + (D_l - C_l)
    for dst, ver in ((Cd1, 0), (Cd2, 2)):
        nc.vector.tensor_tensor(out=dst[:], in0=Rv(3, ver), in1=Rv(2, ver),
                                op=OP.subtract)
        t2 = tmp("cdt")
        nc.vector.tensor_tensor(out=t2[:], in0=Rv(3, ver + 1),
                                in1=Rv(2, ver + 1), op=OP.subtract)
        nc.vector.tensor_tensor(out=dst[:], in0=dst[:], in1=t2[:], op=OP.add)

    blk = cp.tile([128, 128], U8, tag="blk", name="t")
    nc.gpsimd.tensor_scalar(out=blk[:], in0=tqT[:], scalar1=C1, scalar2=None,
                            op0=OP.is_ge)
    biasC1 = cp.tile([128, 1], F32, tag="biasC1", name="t")
    nc.vector.memset(biasC1[:], C1)
    biasC2 = cp.tile([128, 1], F32, tag="biasC2", name="t")
    nc.vector.memset(biasC2[:], C2)
    e1 = tmp("e1")
    nc.scalar.activation(e1[:], tqT[:], ACTF.Exp, bias=biasC1[:], scale=-1.0)
    e2 = tmp("e2")
    nc.scalar.activation(e2[:], tqT[:], ACTF.Exp, bias=biasC2[:], scale=-1.0)

    def sel(tag, on_true, on_false):
        o = tmp(tag)
        nc.vector.select(o[:], blk[:], on_true[:], on_false[:])
        return o

    esel = sel("esel", e2, e1)
    Asel = sel("Asel", A2, A1)
    Bsel = sel("Bsel", B2, B1)
    CDsel = sel("CDsel", Cd2, Cd1)

    feat0 = tmp("feat0")
    nc.vector.tensor_tensor(out=feat0[:], in0=esel[:], in1=Asel[:], op=OP.mult)
    nc.vector.tensor_tensor(out=feat0[:], in0=feat0[:], in1=esel[:],
                            op=OP.mult)
    feat1 = tmp("feat1")
    nc.vector.tensor_tensor(out=feat1[:], in0=esel[:], in1=Bsel[:], op=OP.mult)
    feat2 = tmp("feat2")
    nc.vector.tensor_tensor(out=feat2[:], in0=CDsel[:], in1=esel[:],
                            op=OP.mult)

    # eff0 = esc - 2*E'' : head cols direct from states, grid from stage
    eff0 = tmp("eff0")
    nc.gpsimd.tensor_scalar(out=eff0[:, 0:32], in0=ST[4, 0],
                            scalar1=-2.0, scalar2=1.0, op0=OP.mult, op1=OP.add)
    nc.gpsimd.tensor_scalar(out=eff0[:, 32:64], in0=ST[4, 1],
                            scalar1=-2.0, scalar2=1.0, op0=OP.mult, op1=OP.add)
    nc.vector.scalar_tensor_tensor(out=eff0[:, 64:128], in0=Ev(),
                                   scalar=-2.0, in1=esc_ps[:, 64:128],
                                   op0=OP.mult, op1=OP.add)

    combo = tmp("combo")
    nc.vector.tensor_scalar(out=combo[:], in0=feat0[:], scalar1=wbbc[:, 0:1],
                            scalar2=None, op0=OP.mult)
    nc.vector.scalar_tensor_tensor(out=combo[:], in0=feat1[:],
                                   scalar=wbbc[:, 1:2], in1=combo[:],
                                   op0=OP.mult, op1=OP.add)
    nc.vector.scalar_tensor_tensor(out=combo[:], in0=feat2[:],
                                   scalar=negw2[:], in1=combo[:],
                                   op0=OP.mult, op1=OP.add)
    logits = tmp("logits")
    nc.vector.tensor_tensor(out=logits[:], in0=combo[:], in1=eff0[:],
                            op=OP.mult)
    nc.vector.tensor_scalar(out=logits[:], in0=logits[:], scalar1=wbbc[:, 3:4],
                            scalar2=None, op0=OP.add)
    # zero the pad query (qt1 block, q = 127): engine ops must start at a
    # 32-aligned partition, so overwrite via DMA from a zero row instead.
    zrow = cp.tile([1, 32], F32, tag="zrow", name="t")
    nc.vector.memset(zrow[:], 0.0)
    nc.sync.dma_start(out=logits[127:128, 32:64], in_=zrow[:])

    if dbg_d is not None:
        nc.sync.dma_start(out=dbg_d[:, 0:128], in_=logits[:])
        nc.sync.dma_start(out=dbg_d[:, 128:256], in_=eff0[:])
        nc.sync.dma_start(out=nc._dbg2[:], in_=stage[:])

    expg = cp.tile([128, 64], F32, tag="expg", name="t")
    nc.scalar.activation(expg[:], logits[:, 64:128], ACTF.Exp)

    # partition reductions via ones matmuls
    red_ps = pf.tile([64, 2], F32, tag="redps", name="t")
    nc.tensor.matmul(red_ps[:, 0:1], logits[:, 0:64], onescol[:],
                     start=True, stop=True)
    nc.tensor.matmul(red_ps[:, 1:2], expg[:], onescol[:],
                     start=True, stop=True)
    red = cp.tile([64, 2], F32, tag="red", name="t")
    nc.scalar.copy(red[:], red_ps[:])
    comb = cp.tile([64, 1], F32, tag="comb", name="t")
    nc.vector.tensor_scalar(out=comb[:], in0=red[:, 1:2], scalar1=-RES,
                            scalar2=None, op0=OP.mult)
    nc.vector.tensor_tensor(out=comb[:], in0=comb[:], in1=red[:, 0:1],
                            op=OP.add)
    fin_ps = pf.tile([1, 1], F32, tag="finps", name="t")
    nc.tensor.matmul(fin_ps[:], comb[:], onescol[0:64, :], start=True,
                     stop=True)
    fin = cp.tile([1, 1], F32, tag="fin", name="t")
    nc.vector.tensor_copy(out=fin[:], in_=fin_ps[:])
    nc.sync.dma_start(out=out_d[:], in_=fin[:])

    pf.release()
    for pool in (imp, map_, mp, qp, sp, cp):
        pool.release()


_NC_CACHE = []


def _get_nc():
    if not _NC_CACHE:
        _NC_CACHE.append(build_nc())
    return _NC_CACHE[0]


def make_inputs_for_core(times, states, base, weights, core):
    f32 = np.float32
    sl = slice(core * S, (core + 1) * S)
    t = np.ascontiguousarray(times[sl]).astype(f32)
    st = np.ascontiguousarray(states[sl]).astype(np.int32)
    grid = np.arange(0.0, T_MAX, RES, dtype=f32)
    head = t[:, 4, :]
    q_raw = np.concatenate(
        [head[:, 1:256], head[:, 255:256], np.tile(grid, (S, 1))],
        axis=1).astype(f32)
    tqp = (q_raw - f32(TOL)).astype(f32)
    tqinner = (t[:, 1, :] - f32(TOL)).astype(f32)
    tqmain = np.concatenate([tqp, q_raw, tqinner], axis=1).astype(f32)

    # prep [128, 1088]: host-pretransposed event data
    #  cols    0- 319: T  (a,kt) blocks of S: t[:, a, kt*128+p].T
    #  cols  320- 575: Tpk for a in (0,1,2,4): interleaved cols 2s+kt
    #  cols  576- 703: -K*t3[kt0], -K*t3[kt1], -K*t1[kt0], -K*t1[kt1]
    #  cols  704-1023: ST (a,kt) blocks (states as f32)
    #  cols 1024-1087: shm1[kt]: sh_{j-1} with wrap
    prep = np.zeros((128, 1280), f32)
    stf = st.astype(f32)
    for a in range(5):
        for kt in range(2):
            b = (2 * a + kt) * S
            prep[:, b:b + S] = t[:, a, kt * EH:(kt + 1) * EH].T
            prep[:, 704 + b:704 + b + S] = stf[:, a, kt * EH:(kt + 1) * EH].T
    for ai, a in enumerate((0, 1, 2, 4)):
        x = t[:, a, :].reshape(S, 2, EH)          # [s, kt, p]
        prep[:, 320 + ai * 2 * S:320 + (ai + 1) * 2 * S] = \
            np.transpose(x, (2, 0, 1)).reshape(EH, 2 * S)
    for j, (a, kt) in enumerate(((3, 0), (3, 1), (1, 0), (1, 1))):
        prep[:, 576 + j * S:576 + (j + 1) * S] = \
            (t[:, a, kt * EH:(kt + 1) * EH].T * f32(-BIGK))
    shr = np.roll(st[:, 4, :], 1, axis=1).astype(f32)
    for kt in range(2):
        prep[:, 1024 + kt * S:1024 + (kt + 1) * S] = \
            shr[:, kt * EH:(kt + 1) * EH].T
    # Tpk21: [t2k0, t2k1, t1k0, t1k1] interleaved at col 4s
    q21 = np.stack([t[:, 2, 0:EH], t[:, 2, EH:E],
                    t[:, 1, 0:EH], t[:, 1, EH:E]], axis=2)  # [s, p, 4]
    prep[:, 1088:1216] = np.transpose(q21, (1, 0, 2)).reshape(EH, 4 * S)
    for kt in range(2):
        prep[:, 1216 + kt * S:1216 + (kt + 1) * S] = \
            (t[:, 0, kt * EH:(kt + 1) * EH].T * f32(-BIGK))

    esrow1 = np.ones((1, 128), f32)
    es = (1.0 - 2.0 * st[:, 4, 255]).astype(f32)
    esrow1[0, 64:96] = es
    esrow1[0, 96:128] = es

    return {
        "times": t,
        "states": st,
        "base": np.asarray(base, f32),
        "weights": np.asarray(weights, f32),
        "tqmain": np.ascontiguousarray(tqmain),
        "eye32": np.eye(32, dtype=f32),
        "prep": prep,
        "esrow1": esrow1,
    }


def kernel(times, states, base, weights):
    from concourse.bass_utils import run_bass_kernel_spmd

    times = np.asarray(times, np.float32)
    states = np.asarray(states, np.int32)
    nc = _get_nc()
    in_maps = [make_inputs_for_core(times, states, base, weights, c)
               for c in range(NCORES)]
    res = run_bass_kernel_spmd(nc, in_maps, list(range(NCORES)))
    total = np.float32(0.0)
    for c in range(NCORES):
        total += np.asarray(res.results[c]["out"], np.float32)[0]
    return np.array([total], dtype=np.float32)


def run_traced(times, states, base, weights):
    """Profiled run; returns HW exec time in ns (or None if tracing off)."""
    from concourse.bass_utils import run_bass_kernel_spmd

    times = np.asarray(times, np.float32)
    states = np.asarray(states, np.int32)
    nc = _get_nc()
    in_maps = [make_inputs_for_core(times, states, base, weights, c)
               for c in range(NCORES)]
    res = run_bass_kernel_spmd(nc, in_maps, list(range(NCORES)), trace=True)
    return res.exec_time_ns


# revision 7
# speedup vs baseline: 1.1683x; 1.1683x over previous
"""Trainium2 Bass kernel for nn_Logic_Learning_Model (logic-rule point-process
log-likelihood), restructured for speed.

Key differences vs the first-generation kernel:
- No per-partition-scalar (imm_src=PTR) tensor_scalar ops: those run at
  ~15 ns/elem on HW. Masks are built with DVE tensor_tensor against
  stride-0 broadcast threshold columns (exact 0/1) and with ScalarE
  Sigmoid(K*(tq - t)) with K=1e30 (saturates to exact 0/1 in fp32).
- Mask matmuls are flipped: mask is the lhsT (stationary), value quads are
  the rhs, so PSUM comes out query-major [q, role]. This eliminates the
  partition->free repack DMAs (82K 4-byte packets) of the old kernel.
- Query rows (head/pad/grid, and the tq-TOL variant with reference fp32
  rounding) are precomputed on the host and broadcast by one DMA per
  sample.
- The E (head-state) role needs no matmul for head queries: sh[idx] for
  head query i is states[s,4,i-1], already laid out [event, sample] in
  SBUF. Only the 256 grid queries use a mask matmul.
- Final reduction (sum of head logits, sum of exp(grid logits)) happens
  on-device via ones-matmuls; each core returns one scalar.

Sharding: pure data parallel, 32 samples per core on 8 cores; host sums
the 8 per-core scalars.
"""

import numpy as np

import concourse.bass as bass
import concourse.mybir as mybir
from concourse.tile import TileContext

F32 = mybir.dt.float32
BF16 = mybir.dt.bfloat16
I32 = mybir.dt.int32
U8 = mybir.dt.uint8

NCORES = 8
S = 32          # samples per core
E = 256         # events per predicate
EH = 128        # one partition tile of events
Q = 512         # queries: 255 head + 1 pad + 256 grid
QM = 1024       # tq cols: [fl(tq-0.1) | tq]
QT = 1280       # tqmain row width: [fl(tq-0.1) | tq | fl(t1-0.1)]
T_MAX = 76.8
RES = 0.3
TOL = 0.1
C1 = 38.4
C2 = 76.8
BIGK = 1.0e30   # sigmoid sharpness: saturates to exact 0/1 in fp32
ASGN = 22       # ~2/3 of samples: A-masks on ScalarE, interleaved

OP = mybir.AluOpType
ACTF = mybir.ActivationFunctionType


def bcast(ap, n=128):
    """0-stride partition broadcast view of a flat DRAM AP."""
    return bass.AP(ap.tensor, ap.offset, [[0, n]] + list(ap.ap))


def pk_in0(ap, colw):
    """[128, colw] view read twice -> [128, 2*colw] (outer stride 0)."""
    return bass.AP(ap.tensor, ap.offset, [list(ap.ap[0]), [0, 2], [1, colw]])


def pk_in1(ap, colw):
    """[128, 2] column pair -> [128, 2*colw] (inner stride 0)."""
    return bass.AP(ap.tensor, ap.offset, [list(ap.ap[0]), [1, 2], [0, colw]])


def pk4_in0(ap, colw):
    """[128, colw] view read 4x -> [128, 4*colw] (outer stride 0)."""
    return bass.AP(ap.tensor, ap.offset, [list(ap.ap[0]), [0, 4], [1, colw]])


def pk4_in1(ap, colw):
    """[128, 4] column quad -> [128, 4*colw] (inner stride 0)."""
    return bass.AP(ap.tensor, ap.offset, [list(ap.ap[0]), [1, 4], [0, colw]])


def col_bc(ap, colw):
    """[128, 1] column -> [128, colw] stride-0 broadcast."""
    return bass.AP(ap.tensor, ap.offset, [list(ap.ap[0]), [0, colw]])


def build_nc():
    from concourse.bacc import Bacc
    nc = Bacc(None, target_bir_lowering=False)
    times_d = nc.dram_tensor("times", [S, 5, E], F32, kind="ExternalInput")
    states_d = nc.dram_tensor("states", [S, 5, E], I32, kind="ExternalInput")
    base_d = nc.dram_tensor("base", [1], F32, kind="ExternalInput")
    weights_d = nc.dram_tensor("weights", [3], F32, kind="ExternalInput")
    tqmain_d = nc.dram_tensor("tqmain", [S, QT], F32, kind="ExternalInput")
    eye_d = nc.dram_tensor("eye32", [32, 32], F32, kind="ExternalInput")
    # host-pretransposed event data; see make_inputs_for_core for layout
    prep_d = nc.dram_tensor("prep", [128, 1280], F32, kind="ExternalInput")
    esrow_d = nc.dram_tensor("esrow1", [1, 128], F32, kind="ExternalInput")
    out_d = nc.dram_tensor("out", [1], F32, kind="ExternalOutput")
    import os
    dbg_d = None
    if os.environ.get("KDBG"):
        dbg_d = nc.dram_tensor("dbg", [128, 256], F32, kind="ExternalOutput")
        dbg2_d = nc.dram_tensor("dbg2", [128, 2380], F32,
                                kind="ExternalOutput")
        nc._dbg2 = dbg2_d

    with TileContext(nc) as tc:
        _build(tc, nc, times_d, states_d, base_d, weights_d, tqmain_d,
               eye_d, prep_d, esrow_d, out_d, dbg_d)
    nc.finalize()
    return nc


def _build(tc, nc, times_d, states_d, base_d, weights_d, tqmain_d,
           eye_d, prep_d, esrow_d, out_d, dbg_d=None):
    cp = tc.alloc_tile_pool(name="const", bufs=1)
    sp = tc.alloc_tile_pool(name="scratch", bufs=3)
    qp = tc.alloc_tile_pool(name="tq", bufs=4)
    mp = tc.alloc_tile_pool(name="mask", bufs=6)
    map_ = tc.alloc_tile_pool(name="maska", bufs=4)
    imp = tc.alloc_tile_pool(name="maski", bufs=4)
    pw = tc.alloc_tile_pool(name="psw", bufs=1, space="PSUM")
    pr = tc.alloc_tile_pool(name="prole", bufs=1, space="PSUM")

    # ---------------- phase 0: one dense load + batched prep ----------------
    prep = cp.tile([128, 1280], F32, tag="prep", name="t")
    nc.sync.dma_start(out=prep[:], in_=prep_d[:])

    # views into host-pretransposed prep (see make_inputs_for_core)
    T = {(a, kt): prep[:, (2 * a + kt) * S:(2 * a + kt + 1) * S]
         for a in range(5) for kt in range(2)}
    Tpk = {a: prep[:, 320 + ai * 2 * S:320 + (ai + 1) * 2 * S]
           for ai, a in enumerate((0, 1, 2, 4))}
    nKT = {(3, 0): prep[:, 576:608], (3, 1): prep[:, 608:640],
           (1, 0): prep[:, 640:672], (1, 1): prep[:, 672:704]}
    ST = {(a, kt): prep[:, 704 + (2 * a + kt) * S:704 + (2 * a + kt + 1) * S]
          for a in range(5) for kt in range(2)}
    shm1 = {kt: prep[:, 1024 + kt * S:1024 + (kt + 1) * S] for kt in range(2)}
    Tpk21 = prep[:, 1088:1216]   # [t2k0,t2k1,t1k0,t1k1] at 4s
    nKT0 = {kt: prep[:, 1216 + kt * S:1216 + (kt + 1) * S]
            for kt in range(2)}

    # base/weights broadcast columns
    wbbc = cp.tile([128, 4], F32, tag="wbbc", name="t")
    nc.vector.memset(wbbc[:], 0.0)
    nc.sync.dma_start(out=wbbc[:, 0:3], in_=bcast(weights_d[:]))
    nc.sync.dma_start(out=wbbc[:, 3:4], in_=bcast(base_d[:]))
    negw2 = cp.tile([128, 1], F32, tag="negw2", name="t")
    nc.vector.tensor_scalar(out=negw2[:], in0=wbbc[:, 2:3], scalar1=-1.0,
                            scalar2=None, op0=OP.mult)

    # batched exponentials / state masks
    ew = {}
    sm = {}
    for kt in range(2):
        def _exp(tag, src, scale, off):
            arg = sp.tile([EH, S], F32, tag=f"arg{tag}{kt}", name="t")
            nc.vector.tensor_scalar(out=arg[:], in0=src, scalar1=scale,
                                    scalar2=off, op0=OP.mult, op1=OP.add)
            e_t = cp.tile([EH, S], F32, tag=f"e{tag}{kt}", name="t")
            nc.scalar.activation(e_t[:], arg[:], ACTF.Exp)
            return e_t

        ew["w0", kt] = _exp("w0", T[0, kt], 1.0, -C2)        # e^{t0-C2}
        ew["c2t1", kt] = _exp("c2t1", T[1, kt], -1.0, C2)    # e^{C2-t1}
        ew["g1", kt] = _exp("g1", T[1, kt], 2.0, -2.0 * C1)  # e^{2(t1-C1)}
        ew["g2", kt] = _exp("g2", T[1, kt], 2.0, -2.0 * C2)
        ew["v21", kt] = _exp("v21", T[2, kt], 1.0, -C1)
        ew["v22", kt] = _exp("v22", T[2, kt], 1.0, -C2)
        ew["v31", kt] = _exp("v31", T[3, kt], 1.0, -C1)
        ew["v32", kt] = _exp("v32", T[3, kt], 1.0, -C2)

        for a, val, tag in ((0, 1, "s0"), (1, 1, "s1"), (2, 1, "s2"), (3, 0, "s3")):
            m = cp.tile([EH, S], F32, tag=f"{tag}{kt}", name="t")
            nc.vector.tensor_scalar(out=m[:], in0=ST[a, kt], scalar1=val,
                                    scalar2=None, op0=OP.is_equal)
            sm[tag, kt] = m

        # [t3 <= C1]: zero the v3C1 entries no C1-block query can select,
        # keeping the C1-version value range bounded.
        m31 = cp.tile([EH, S], F32, tag=f"m31{kt}", name="t")
        nc.vector.tensor_scalar(out=m31[:], in0=T[3, kt], scalar1=C1,
                                scalar2=None, op0=OP.is_le)
        sm["m31", kt] = m31

    def dekker(dst, blk0, src32, tmp_tag):
        """bf16 (hi, lo) of src32 [128, S] into dst col blocks blk0, blk0+1."""
        hi = dst[:, blk0 * S:(blk0 + 1) * S]
        lo = dst[:, (blk0 + 1) * S:(blk0 + 2) * S]
        nc.vector.tensor_copy(out=hi, in_=src32[:])
        tmp = sp.tile([EH, S], F32, tag=tmp_tag, name="t")
        nc.vector.tensor_copy(out=tmp[:], in_=hi)
        nc.vector.tensor_tensor(out=lo, in0=src32[:], in1=tmp[:],
                                op=OP.subtract)

    # w0 pairs: [128, 2S], blocks [hi | lo]
    w0pair = {}
    for kt in range(2):
        w0 = sp.tile([EH, S], F32, tag=f"w0m{kt}", name="t")
        nc.vector.tensor_tensor(out=w0[:], in0=ew["w0", kt][:],
                                in1=sm["s0", kt][:], op=OP.mult)
        pair = cp.tile([EH, 2 * S], BF16, tag=f"w0pair{kt}", name="t")
        dekker(pair, 0, w0, f"w0tmp{kt}")
        w0pair[kt] = pair

    # vB / vC quads [128, 4S]: blocks [v1h | v1l | v2h | v2l]
    vB = {}
    vC = {}
    for kt in range(2):
        q_b = cp.tile([EH, 4 * S], BF16, tag=f"vB{kt}", name="t")
        q_c = cp.tile([EH, 4 * S], BF16, tag=f"vC{kt}", name="t")
        for ver, (e2tag, e3tag) in enumerate((("v21", "v31"), ("v22", "v32"))):
            v2 = sp.tile([EH, S], F32, tag=f"v2m{kt}{ver}", name="t")
            nc.vector.tensor_tensor(out=v2[:], in0=ew[e2tag, kt][:],
                                    in1=sm["s2", kt][:], op=OP.mult)
            dekker(q_b, 2 * ver, v2, f"dkb{kt}{ver}")
            v3 = sp.tile([EH, S], F32, tag=f"v3m{kt}{ver}", name="t")
            nc.vector.tensor_tensor(out=v3[:], in0=ew[e3tag, kt][:],
                                    in1=sm["s3", kt][:], op=OP.mult)
            if ver == 0:
                nc.vector.tensor_tensor(out=v3[:], in0=v3[:],
                                        in1=sm["m31", kt][:], op=OP.mult)
            dekker(q_c, 2 * ver, v3, f"dkc{kt}{ver}")
        vB[kt] = q_b
        vC[kt] = q_c

    # dsh (bf16) [128, S]: sh_j - sh_{j-1 (wrap)} (shifted sh from prep)
    dsh = {}
    for kt in range(2):
        d = cp.tile([EH, S], BF16, tag=f"dsh{kt}", name="t")
        nc.vector.tensor_tensor(out=d[:], in0=ST[4, kt], in1=shm1[kt],
                                op=OP.subtract)
        dsh[kt] = d

    # escrow [1, 128]: head cols 1.0; grid cols 1 - 2*sh255 (host-built)
    escrow = cp.tile([1, 128], F32, tag="escrow", name="t")
    nc.sync.dma_start(out=escrow[:], in_=esrow_d[:])

    ones1 = cp.tile([1, 128], F32, tag="ones1", name="t")
    nc.vector.memset(ones1[:], 1.0)
    onescol = cp.tile([128, 1], F32, tag="onescol", name="t")
    nc.vector.memset(onescol[:], 1.0)

    eye32 = cp.tile([32, 32], F32, tag="eye32", name="t")
    nc.sync.dma_start(out=eye32[:], in_=eye_d[:])
    tqrows = cp.tile([32, Q], F32, tag="tqrows", name="t")
    nc.sync.dma_start(out=tqrows[:], in_=tqmain_d[:, Q:QM])

    # ------------- merged main loop (groups of 4 samples) -------------
    # Per sample: inner mask + all role masks + inner/BCDE matmuls. After
    # each group's inner matmuls: per-group gA assembly, then the group's
    # A-role matmuls. This keeps DVE/ScalarE/PE busy concurrently instead
    # of a serial inner-mask prefix.
    # One PSUM bank per role (matmul start=True clears has_written for the
    # whole 2KB zero region -> one open accumulation group per bank).
    # ABCD: col = s*16 + qt*4; E: col = s*4 + qt.
    psw = pw.tile([128, 4 * S], F32, tag="psw", name="t")
    psA = pr.tile([128, 512], F32, tag="psA", name="t")
    psB = pr.tile([128, 512], F32, tag="psB", name="t")
    psC = pr.tile([128, 512], F32, tag="psC", name="t")
    psD = pr.tile([128, 512], F32, tag="psD", name="t")
    psE = pr.tile([128, 128], F32, tag="psE", name="t")

    wst = cp.tile([128, 4 * S], F32, tag="wst", name="t")
    gA = {kt: cp.tile([EH, 4 * S], BF16, tag=f"gA{kt}", name="t")
          for kt in range(2)}

    def build_masks(s, tq4, j, inner=True):
        tqm = tq4[:, j * QM:(j + 1) * QM]
        im = None
        mEg = mp.tile([EH, Q], BF16, tag="mEg", name="t")
        nc.vector.tensor_tensor(out=mEg[:], in0=pk_in0(tqm[:, 768:QM], 256),
                                in1=pk_in1(Tpk[4][:, 2 * s:2 * s + 2], 256),
                                op=OP.is_gt)
        mCD = {}
        for kt in range(2):
            cd = mp.tile([EH, QM], BF16, tag=f"mCD{kt}", name="t")
            nc.scalar.activation(cd[:], tqm, ACTF.Sigmoid,
                                 bias=nKT[3, kt][:, s:s + 1], scale=BIGK)
            mCD[kt] = cd
        if s % 2 == 0:  # A on ScalarE; B alone on DVE
            mB = mp.tile([EH, QM], BF16, tag="mB", name="t")
            nc.vector.tensor_tensor(out=mB[:], in0=pk_in0(tqm[:, 0:Q], Q),
                                    in1=pk_in1(Tpk[2][:, 2 * s:2 * s + 2], Q),
                                    op=OP.is_gt)
            mA = map_.tile([EH, QM], BF16, tag="mA", name="t")
            for kt in range(2):
                nc.scalar.activation(mA[:, kt * Q:(kt + 1) * Q], tqm[:, 0:Q],
                                     ACTF.Sigmoid, bias=nKT[1, kt][:, s:s + 1],
                                     scale=BIGK)
        else:  # one packed DVE op: [mB-kt0 | mB-kt1 | mA-kt0 | mA-kt1]
            ba = map_.tile([EH, 2 * QM], BF16, tag="mBA", name="t")
            nc.vector.tensor_tensor(out=ba[:], in0=pk4_in0(tqm[:, 0:Q], Q),
                                    in1=pk4_in1(Tpk21[:, 4 * s:4 * s + 4], Q),
                                    op=OP.is_gt)
            mB = ba[:, 0:QM]
            mA = ba[:, QM:2 * QM]
        return im, mB, mEg, mCD, mA

    def inner_mms(s, im):
        for jkt in range(2):
            for ikt in range(2):
                nc.tensor.matmul(
                    psw[:, 4 * s + 2 * jkt:4 * s + 2 * jkt + 2],
                    im[:, ikt * E + jkt * EH:ikt * E + (jkt + 1) * EH],
                    w0pair[ikt][:, s::S][:, 0:2],
                    start=(ikt == 0), stop=(ikt == 1))

    def bcde_mms(s, mB, mEg, mCD):
        for qt in range(4):
            q0 = qt * EH
            c4 = s * 16 + qt * 4
            for kt in range(2):
                nc.tensor.matmul(psB[:, c4:c4 + 4],
                                 mB[:, kt * Q + q0:kt * Q + q0 + EH],
                                 vB[kt][:, s::S][:, 0:4],
                                 start=(kt == 0), stop=(kt == 1))
            for kt in range(2):
                nc.tensor.matmul(psC[:, c4:c4 + 4],
                                 mCD[kt][:, q0:q0 + EH],
                                 vC[kt][:, s::S][:, 0:4],
                                 start=(kt == 0), stop=(kt == 1))
            for kt in range(2):
                nc.tensor.matmul(psD[:, c4:c4 + 4],
                                 mCD[kt][:, Q + q0:Q + q0 + EH],
                                 vC[kt][:, s::S][:, 0:4],
                                 start=(kt == 0), stop=(kt == 1))
            if qt >= 2:
                for kt in range(2):
                    nc.tensor.matmul(
                        psE[:, s * 4 + qt:s * 4 + qt + 1],
                        mEg[:, kt * 256 + (qt - 2) * EH:
                            kt * 256 + (qt - 1) * EH],
                        dsh[kt][:, s:s + 1], start=(kt == 0), stop=(kt == 1))

    def a_mms(s, mA):
        for qt in range(4):
            q0 = qt * EH
            c4 = s * 16 + qt * 4
            for kt in range(2):
                nc.tensor.matmul(psA[:, c4:c4 + 4],
                                 mA[:, kt * Q + q0:kt * Q + q0 + EH],
                                 gA[kt][:, s::S][:, 0:4],
                                 start=(kt == 0), stop=(kt == 1))

    def phase2_half(h):
        """gA columns for samples 16h..16h+15 (batched FD=16)."""
        c0 = 64 * h
        s0 = 16 * h
        nc.scalar.copy(wst[:, c0:c0 + 64], psw[:, c0:c0 + 64])
        for kt in range(2):
            wh = sp.tile([EH, 16], F32, tag=f"wh{kt}", name="t", bufs=3)
            nc.vector.tensor_tensor(out=wh[:],
                                    in0=wst[:, c0 + 2 * kt:c0 + 64:4],
                                    in1=wst[:, c0 + 2 * kt + 1:c0 + 64:4],
                                    op=OP.add)
            nc.vector.tensor_tensor(out=wh[:], in0=wh[:],
                                    in1=ew["c2t1", kt][:, s0:s0 + 16],
                                    op=OP.mult)
            for ver, etag in enumerate(("g1", "g2")):
                g32 = sp.tile([EH, 16], F32, tag=f"g32{kt}{ver}", name="t",
                              bufs=3)
                nc.vector.tensor_tensor(out=g32[:],
                                        in0=ew[etag, kt][:, s0:s0 + 16],
                                        in1=wh[:], op=OP.mult)
                nc.vector.tensor_tensor(out=g32[:], in0=g32[:],
                                        in1=sm["s1", kt][:, s0:s0 + 16],
                                        op=OP.mult)
                blk0 = 2 * ver
                hi = gA[kt][:, blk0 * S + s0:blk0 * S + s0 + 16]
                lo = gA[kt][:, (blk0 + 1) * S + s0:(blk0 + 1) * S + s0 + 16]
                nc.vector.tensor_copy(out=hi, in_=g32[:])
                tmp = sp.tile([EH, 16], F32, tag=f"dkg{kt}{ver}", name="t",
                              bufs=3)
                nc.vector.tensor_copy(out=tmp[:], in_=hi)
                nc.vector.tensor_tensor(out=lo, in0=g32[:], in1=tmp[:],
                                        op=OP.subtract)

    # pass A: inner masks + What matmuls (overlaps pass B via scheduler)
    for g in range(S // 4):
        tqi4 = qp.tile([EH, 4 * E], F32, tag="tqi4", name="t", bufs=2)
        nc.sync.dma_start(out=tqi4[:],
                          in_=bcast(tqmain_d[4 * g:4 * g + 4, QM:QT]))
        for j in range(4):
            s = 4 * g + j
            tqi = tqi4[:, j * E:(j + 1) * E]
            im = imp.tile([EH, 2 * E], BF16, tag="im", name="t")
            nc.vector.tensor_tensor(out=im[:], in0=pk_in0(tqi, E),
                                    in1=pk_in1(Tpk[0][:, 2 * s:2 * s + 2], E),
                                    op=OP.is_gt)
            inner_mms(s, im)
    phase2_half(0)
    phase2_half(1)

    # pass B: role masks + all role matmuls
    stage = cp.tile([128, 2176], F32, tag="stage", name="t")

    def stage_copy_half(h):
        c = 256 * h
        for i, ps in enumerate((psA, psB, psC, psD)):
            nc.scalar.copy(stage[:, i * 512 + c:i * 512 + c + 256],
                           ps[:, c:c + 256])
        nc.scalar.copy(stage[:, 2048 + 64 * h:2048 + 64 * h + 64],
                       psE[:, 64 * h:64 * h + 64])

    for g in range(S // 4):
        tq4 = qp.tile([EH, 4 * QM], F32, tag="tq4", name="t", bufs=4)
        nc.sync.dma_start(out=tq4[:],
                          in_=bcast(tqmain_d[4 * g:4 * g + 4, 0:QM]))
        for j in range(4):
            s = 4 * g + j
            _, mB, mEg, mCD, mA = build_masks(s, tq4, j, inner=False)
            bcde_mms(s, mB, mEg, mCD)
            a_mms(s, mA)
        if g == S // 8 - 1:
            stage_copy_half(0)
    stage_copy_half(1)

    pr.release()
    pw.release()

    # ---------------- phase 4: batched post-processing ----------------
    pf = tc.alloc_tile_pool(name="pfin", bufs=1, space="PSUM")

    def Rv(role, hilo):
        """[128, 4, 32, 1] view of one (role-bank, hilo) plane, (qt, s)."""
        v = stage[:, role * 512:(role + 1) * 512].rearrange(
            "p (s q r) -> p q s r", s=S, q=4, r=4)
        return v[:, :, :, hilo:hilo + 1]

    def Ev():  # E grid half -> [128, 2, 32, 1], (qt-2, s)
        v = stage[:, 2048:2176].rearrange("p (s q) -> p q s", s=S, q=4)
        return v[:, 2:4, :]

    # tqT [128 q, 128 (qt,s)] via transpose matmuls
    tqT_ps = pf.tile([128, 128], F32, tag="tqTps", name="t")
    for qt in range(4):
        nc.tensor.matmul(tqT_ps[:, 32 * qt:32 * (qt + 1)],
                         tqrows[:, qt * EH:(qt + 1) * EH], eye32[:],
                         start=True, stop=True)
    tqT = cp.tile([128, 128], F32, tag="tqT", name="t")
    nc.scalar.copy(tqT[:], tqT_ps[:])

    # esc psum [128, 128] = escrow broadcast down partitions
    esc_ps = pf.tile([128, 128], F32, tag="escps", name="t")
    nc.tensor.matmul(esc_ps[:], ones1[:], escrow[:], start=True, stop=True)

    def tmp(tag):
        return cp.tile([128, 128], F32, tag=tag, name="t")

    # hi+lo pair sums: quad cols per (s,qt) = [v1h, v1l, v2h, v2l]
    A1, A2 = tmp("A1"), tmp("A2")
    B1, B2 = tmp("B1"), tmp("B2")
    Cd1, Cd2 = tmp("Cd1"), tmp("Cd2")
    for dst, role, ver in ((A1, 0, 0), (A2, 0, 2), (B1, 1, 0), (B2, 1, 2)):
        nc.vector.tensor_tensor(out=dst[:], in0=Rv(role, ver),
                                in1=Rv(role, ver + 1), op=OP.add)
    # D - C per version: (D_h - C_h) + (D_l - C_l)
    for dst, ver in ((Cd1, 0), (Cd2, 2)):
        nc.vector.tensor_tensor(out=dst[:], in0=Rv(3, ver), in1=Rv(2, ver),
                                op=OP.subtract)
        t2 = tmp("cdt")
        nc.vector.tensor_tensor(out=t2[:], in0=Rv(3, ver + 1),
                                in1=Rv(2, ver + 1), op=OP.subtract)
        nc.vector.tensor_tensor(out=dst[:], in0=dst[:], in1=t2[:], op=OP.add)

    blk = cp.tile([128, 128], U8, tag="blk", name="t")
    nc.gpsimd.tensor_scalar(out=blk[:], in0=tqT[:], scalar1=C1, scalar2=None,
                            op0=OP.is_ge)
    biasC1 = cp.tile([128, 1], F32, tag="biasC1", name="t")
    nc.vector.memset(biasC1[:], C1)
    biasC2 = cp.tile([128, 1], F32, tag="biasC2", name="t")
    nc.vector.memset(biasC2[:], C2)
    e1 = tmp("e1")
    nc.scalar.activation(e1[:], tqT[:], ACTF.Exp, bias=biasC1[:], scale=-1.0)
    e2 = tmp("e2")
    nc.scalar.activation(e2[:], tqT[:], ACTF.Exp, bias=biasC2[:], scale=-1.0)

    def sel(tag, on_true, on_false):
        o = tmp(tag)
        nc.vector.select(o[:], blk[:], on_true[:], on_false[:])
        return o

    esel = sel("esel", e2, e1)
    Asel = sel("Asel", A2, A1)
    Bsel = sel("Bsel", B2, B1)
    CDsel = sel("CDsel", Cd2, Cd1)

    feat0 = tmp("feat0")
    nc.vector.tensor_tensor(out=feat0[:], in0=esel[:], in1=Asel[:], op=OP.mult)
    nc.vector.tensor_tensor(out=feat0[:], in0=feat0[:], in1=esel[:],
                            op=OP.mult)
    feat1 = tmp("feat1")
    nc.vector.tensor_tensor(out=feat1[:], in0=esel[:], in1=Bsel[:], op=OP.mult)
    feat2 = tmp("feat2")
    nc.vector.tensor_tensor(out=feat2[:], in0=CDsel[:], in1=esel[:],
                            op=OP.mult)

    # eff0 = esc - 2*E'' : head cols direct from states, grid from stage
    eff0 = tmp("eff0")
    nc.gpsimd.tensor_scalar(out=eff0[:, 0:32], in0=ST[4, 0],
                            scalar1=-2.0, scalar2=1.0, op0=OP.mult, op1=OP.add)
    nc.gpsimd.tensor_scalar(out=eff0[:, 32:64], in0=ST[4, 1],
                            scalar1=-2.0, scalar2=1.0, op0=OP.mult, op1=OP.add)
    nc.vector.scalar_tensor_tensor(out=eff0[:, 64:128], in0=Ev(),
                                   scalar=-2.0, in1=esc_ps[:, 64:128],
                                   op0=OP.mult, op1=OP.add)

    combo = tmp("combo")
    nc.vector.tensor_scalar(out=combo[:], in0=feat0[:], scalar1=wbbc[:, 0:1],
                            scalar2=None, op0=OP.mult)
    nc.vector.scalar_tensor_tensor(out=combo[:], in0=feat1[:],
                                   scalar=wbbc[:, 1:2], in1=combo[:],
                                   op0=OP.mult, op1=OP.add)
    nc.vector.scalar_tensor_tensor(out=combo[:], in0=feat2[:],
                                   scalar=negw2[:], in1=combo[:],
                                   op0=OP.mult, op1=OP.add)
    logits = tmp("logits")
    nc.vector.tensor_tensor(out=logits[:], in0=combo[:], in1=eff0[:],
                            op=OP.mult)
    nc.vector.tensor_scalar(out=logits[:], in0=logits[:], scalar1=wbbc[:, 3:4],
                            scalar2=None, op0=OP.add)
    # zero the pad query (qt1 block, q = 127): engine ops must start at a
    # 32-aligned partition, so overwrite via DMA from a zero row instead.
    zrow = cp.tile([1, 32], F32, tag="zrow", name="t")
    nc.vector.memset(zrow[:], 0.0)
    nc.sync.dma_start(out=logits[127:128, 32:64], in_=zrow[:])

    if dbg_d is not None:
        nc.sync.dma_start(out=dbg_d[:, 0:128], in_=logits[:])
        nc.sync.dma_start(out=dbg_d[:, 128:256], in_=eff0[:])
        nc.sync.dma_start(out=nc._dbg2[:], in_=stage[:])

    expg = cp.tile([128, 64], F32, tag="expg", name="t")
    nc.scalar.activation(expg[:], logits[:, 64:128], ACTF.Exp)

    # partition reductions via ones matmuls
    red_ps = pf.tile([64, 2], F32, tag="redps", name="t")
    nc.tensor.matmul(red_ps[:, 0:1], logits[:, 0:64], onescol[:],
                     start=True, stop=True)
    nc.tensor.matmul(red_ps[:, 1:2], expg[:], onescol[:],
                     start=True, stop=True)
    red = cp.tile([64, 2], F32, tag="red", name="t")
    nc.scalar.copy(red[:], red_ps[:])
    comb = cp.tile([64, 1], F32, tag="comb", name="t")
    nc.vector.tensor_scalar(out=comb[:], in0=red[:, 1:2], scalar1=-RES,
                            scalar2=None, op0=OP.mult)
    nc.vector.tensor_tensor(out=comb[:], in0=comb[:], in1=red[:, 0:1],
                            op=OP.add)
    fin_ps = pf.tile([1, 1], F32, tag="finps", name="t")
    nc.tensor.matmul(fin_ps[:], comb[:], onescol[0:64, :], start=True,
                     stop=True)
    fin = cp.tile([1, 1], F32, tag="fin", name="t")
    nc.vector.tensor_copy(out=fin[:], in_=fin_ps[:])
    nc.sync.dma_start(out=out_d[:], in_=fin[:])

    pf.release()
    for pool in (imp, map_, mp, qp, sp, cp):
        pool.release()


_NC_CACHE = []


def _get_nc():
    if not _NC_CACHE:
        _NC_CACHE.append(build_nc())
    return _NC_CACHE[0]


def make_inputs_for_core(times, states, base, weights, core):
    f32 = np.float32
    sl = slice(core * S, (core + 1) * S)
    t = np.ascontiguousarray(times[sl]).astype(f32)
    st = np.ascontiguousarray(states[sl]).astype(np.int32)
    grid = np.arange(0.0, T_MAX, RES, dtype=f32)
    head = t[:, 4, :]
    q_raw = np.concatenate(
        [head[:, 1:256], head[:, 255:256], np.tile(grid, (S, 1))],
        axis=1).astype(f32)
    tqp = (q_raw - f32(TOL)).astype(f32)
    tqinner = (t[:, 1, :] - f32(TOL)).astype(f32)
    tqmain = np.concatenate([tqp, q_raw, tqinner], axis=1).astype(f32)

    # prep [128, 1088]: host-pretransposed event data
    #  cols    0- 319: T  (a,kt) blocks of S: t[:, a, kt*128+p].T
    #  cols  320- 575: Tpk for a in (0,1,2,4): interleaved cols 2s+kt
    #  cols  576- 703: -K*t3[kt0], -K*t3[kt1], -K*t1[kt0], -K*t1[kt1]
    #  cols  704-1023: ST (a,kt) blocks (states as f32)
    #  cols 1024-1087: shm1[kt]: sh_{j-1} with wrap
    prep = np.zeros((128, 1280), f32)
    stf = st.astype(f32)
    for a in range(5):
        for kt in range(2):
            b = (2 * a + kt) * S
            prep[:, b:b + S] = t[:, a, kt * EH:(kt + 1) * EH].T
            prep[:, 704 + b:704 + b + S] = stf[:, a, kt * EH:(kt + 1) * EH].T
    for ai, a in enumerate((0, 1, 2, 4)):
        x = t[:, a, :].reshape(S, 2, EH)          # [s, kt, p]
        prep[:, 320 + ai * 2 * S:320 + (ai + 1) * 2 * S] = \
            np.transpose(x, (2, 0, 1)).reshape(EH, 2 * S)
    for j, (a, kt) in enumerate(((3, 0), (3, 1), (1, 0), (1, 1))):
        prep[:, 576 + j * S:576 + (j + 1) * S] = \
            (t[:, a, kt * EH:(kt + 1) * EH].T * f32(-BIGK))
    shr = np.roll(st[:, 4, :], 1, axis=1).astype(f32)
    for kt in range(2):
        prep[:, 1024 + kt * S:1024 + (kt + 1) * S] = \
            shr[:, kt * EH:(kt + 1) * EH].T
    # Tpk21: [t2k0, t2k1, t1k0, t1k1] interleaved at col 4s
    q21 = np.stack([t[:, 2, 0:EH], t[:, 2, EH:E],
                    t[:, 1, 0:EH], t[:, 1, EH:E]], axis=2)  # [s, p, 4]
    prep[:, 1088:1216] = np.transpose(q21, (1, 0, 2)).reshape(EH, 4 * S)
    for kt in range(2):
        prep[:, 1216 + kt * S:1216 + (kt + 1) * S] = \
            (t[:, 0, kt * EH:(kt + 1) * EH].T * f32(-BIGK))

    esrow1 = np.ones((1, 128), f32)
    es = (1.0 - 2.0 * st[:, 4, 255]).astype(f32)
    esrow1[0, 64:96] = es
    esrow1[0, 96:128] = es

    return {
        "times": t,
        "states": st,
        "base": np.asarray(base, f32),
        "weights": np.asarray(weights, f32),
        "tqmain": np.ascontiguousarray(tqmain),
        "eye32": np.eye(32, dtype=f32),
        "prep": prep,
        "esrow1": esrow1,
    }


def kernel(times, states, base, weights):
    from concourse.bass_utils import run_bass_kernel_spmd

    times = np.asarray(times, np.float32)
    states = np.asarray(states, np.int32)
    nc = _get_nc()
    in_maps = [make_inputs_for_core(times, states, base, weights, c)
               for c in range(NCORES)]
    res = run_bass_kernel_spmd(nc, in_maps, list(range(NCORES)))
    total = np.float32(0.0)
    for c in range(NCORES):
        total += np.asarray(res.results[c]["out"], np.float32)[0]
    return np.array([total], dtype=np.float32)


def run_traced(times, states, base, weights):
    """Profiled run; returns HW exec time in ns (or None if tracing off)."""
    from concourse.bass_utils import run_bass_kernel_spmd

    times = np.asarray(times, np.float32)
    states = np.asarray(states, np.int32)
    nc = _get_nc()
    in_maps = [make_inputs_for_core(times, states, base, weights, c)
               for c in range(NCORES)]
    res = run_bass_kernel_spmd(nc, in_maps, list(range(NCORES)), trace=True)
    return res.exec_time_ns
